# revision 10
# baseline (speedup 1.0000x reference)
"""Multi-head attention (B=8, H=8, S=1024, d=128) on 8 TRN2 NeuronCores.

Strategy
--------
- Data-parallel over batch: core i handles batch i (8 cores, B=8).
- Host-side prep (layout only): per batch, compact keys/values to the
  seq_mask-selected rows (zero-padded to a multiple of 128 -> kt_tiles
  k-tiles), pre-transpose Q and compacted K so the contraction dim (d)
  lands on SBUF partitions, and cast matmul operands to fp16. V is
  augmented per head with a 129th "indicator" column (1 for real keys,
  0 for padding) so the softmax denominator falls out of the AV matmul.
- Device math per head h:
    logitsT[k, q] = K_h^T.T @ Q_h^T          (PE, M=128 k-tiles, N=512)
    W^T[k, q]     = exp(logitsT * d^-0.5)    (ACT, PSUM -> SBUF fp16,
                                              batched in alternating
                                              1536/1024-col groups to
                                              amortize ACTIVATE overhead)
    out[q, 129]   = sum_kt W^T[kt,qtile].T @ [V_h[kt] | ind[kt]]
                                             (PE, M=128 q-tiles, N=129,
                                              PSUM accumulation over kt;
                                              col 128 = denominator)
    osb[q, d]     = out[:, :128] * recip(out[:, 128])  (DVE)
  The learned scalar bias b cancels in softmax (shift invariance) and
  the -1e30 masking is equivalent to dropping masked keys, which the
  compaction does exactly.
- Software pipelining: AV+epilogue of head h-1 are emitted after QK of
  head h so the scalar engine (the bottleneck at ~42us) never waits at
  head boundaries. A short burst of dummy matmuls at kernel start warms
  the PE HAM clock gate while the first input DMAs are in flight.
- Output per head is DMA'd as a contiguous [128, 1024] fp16 block
  ([q-within-tile, (q-tile, d)]); the host reassembles [S, D] and
  handles the degenerate all-masked batch (uniform average).
"""
from contextlib import ExitStack

import numpy as np

import concourse.bacc as bacc
import concourse.mybir as mybir
import concourse.tile as tile
from concourse.bass_utils import run_bass_kernel_spmd

F32 = mybir.dt.float32
F16 = mybir.dt.float16
Exp = mybir.ActivationFunctionType.Exp
Copy = mybir.ActivationFunctionType.Copy

B, S, D, H = 8, 1024, 1024, 8
DH = D // H              # 128, head dim = one partition tile
SCALE = float(DH) ** -0.5
NQT = S // 128           # 8 q-tiles per head

_NC_CACHE: dict[tuple, object] = {}

# build options (overridable for profiling experiments)
OPTS: dict = {}


def _exp_groups(total):
    """Alternating 1536/1024-col exp groups covering `total` columns."""
    groups, pos, a = [], 0, True
    while pos < total:
        size = min(1536 if a else 1024, total - pos)
        groups.append((a, pos, size))
        pos += size
        a = not a
    return groups


def _build(kt_tiles: int, opts: dict | None = None):
    """Build + compile the per-core kernel for `kt_tiles` 128-wide key tiles."""
    opts = opts or {}
    w_bufs = opts.get("w_bufs", 2)
    o_bufs = opts.get("o_bufs", 2)
    n_warm = opts.get("n_warm", 5)
    KP = kt_tiles * 128
    VW = H * 129             # per-k-row width of augmented V
    TOT = kt_tiles * 1024    # logits columns per head
    groups = _exp_groups(TOT)
    nc = bacc.Bacc("TRN2", target_bir_lowering=False, debug=False)

    q_t = nc.dram_tensor("q_t", [D, S], F16, kind="ExternalInput")
    k_t = nc.dram_tensor("k_t", [D, KP], F16, kind="ExternalInput")
    v_a = nc.dram_tensor("v_a", [KP, VW], F16, kind="ExternalInput")
    out_t = nc.dram_tensor("out_t", [H, 128, S], F16, kind="ExternalOutput")

    # ps_out layout: 3 bank-aligned groups of q-tiles (3+3+2), each q-tile
    # owning 129 columns (128 dims + denominator). Offsets within the
    # [128, 1536] tile; a 129-wide matmul output may not cross a PSUM bank.
    def po_off(qi):
        g, j = divmod(qi, 3)
        return g * 512 + j * 129

    with tile.TileContext(nc) as tc, ExitStack() as ctx:
        sb_k = ctx.enter_context(tc.tile_pool(name="sb_k", bufs=1))
        sb_q = ctx.enter_context(tc.tile_pool(name="sb_q", bufs=1))
        sb_v = ctx.enter_context(tc.tile_pool(name="sb_v", bufs=1))
        sb_wm = ctx.enter_context(tc.tile_pool(name="sb_wm", bufs=1))
        sb_w = ctx.enter_context(tc.tile_pool(name="sb_w", bufs=w_bufs))
        sb_o = ctx.enter_context(tc.tile_pool(name="sb_o", bufs=o_bufs))
        ps_a = ctx.enter_context(tc.tile_pool(name="ps_a", bufs=1, space="PSUM"))
        ps_b = ctx.enter_context(tc.tile_pool(name="ps_b", bufs=1, space="PSUM"))
        ps_o = ctx.enter_context(tc.tile_pool(name="ps_o", bufs=1, space="PSUM"))

        kall = sb_k.tile([128, H * KP], F16)
        qall = sb_q.tile([128, H * S], F16)
        vall = sb_v.tile([128, kt_tiles * VW], F16)

        # --- PE warmup: dense dummy matmuls while the first DMAs fly, so
        # the HAM clock gate reaches 8/8 before real work arrives.
        if n_warm:
            wl = sb_wm.tile([128, 128], F16)
            wr = sb_wm.tile([128, 512], F16)
            nc.gpsimd.memset(wl[:], 0.0)
            nc.gpsimd.memset(wr[:], 0.0)
            warm_po = ps_o.tile([128, 1536], F32, tag="po", name="po_warm")
            for _ in range(n_warm):
                nc.tensor.matmul(warm_po[:, 0:512], wl[:], wr[:],
                                 start=True, stop=True, skip_group_check=True)

        # --- Input DMAs, split per head / per k-tile; the first few are
        # issued from different engines so their transfers overlap.
        def dma_k(h, eng):
            eng.dma_start(
                kall[:, h * KP:(h + 1) * KP], k_t.ap()[h * DH:(h + 1) * DH, :])

        def dma_q(h, eng):
            eng.dma_start(
                qall[:, h * S:(h + 1) * S], q_t.ap()[h * DH:(h + 1) * DH, :])

        def dma_v(kt, eng):
            # straight 2D copy, 2064B rows
            eng.dma_start(
                vall[:, kt * VW:(kt + 1) * VW],
                v_a.ap()[kt * 128:(kt + 1) * 128, :])

        # DMA issue engines: sync/gpsimd/scalar can initiate DMAs. Head 0's
        # first exp group needs k-tiles 0+1 and all of q0, so those three
        # go out in parallel on different engines.
        dma_q(0, nc.sync)
        dma_k(0, nc.gpsimd)
        if kt_tiles > 1:
            dma_k(1, nc.scalar)
        dma_v(0, nc.scalar)
        if kt_tiles > 1:
            dma_v(1, nc.scalar)
        if kt_tiles > 2:
            dma_k(2, nc.gpsimd)
        for kt in range(2, kt_tiles):
            dma_v(kt, nc.gpsimd)
        dma_q(1, nc.sync); dma_q(2, nc.sync)
        for h in range(3, H):
            dma_k(h, nc.gpsimd); dma_q(h, nc.sync)

        # Split the previous head's AV k-tiles across this head's exp
        # groups so the PE fills its exp-wait gaps without head-of-line
        # blocking: emission order per head is
        #   QK(h,g0) exp(h,g0) AV(h-1,kts0) QK(h,g1) exp(h,g1) AV(h-1,kts1) ...
        kt_sets = [[] for _ in groups]
        for kt in range(kt_tiles):
            kt_sets[min(kt * len(groups) // kt_tiles, len(groups) - 1)].append(kt)

        def emit_qk_group(h, gi, ring):
            a, start, size = groups[gi]
            hq = h * S
            pool = ps_a if a else ps_b
            cap = 1536 if a else 1024
            pl = pool.tile([128, cap], F32, tag="pl" + ("A" if a else "B"),
                           name=f"pl_{h}_{start}")
            for local in range(0, size, 512):
                gcol = start + local
                kt, qh = divmod(gcol, 1024)
                lhsT = kall[:, h * KP + kt * 128: h * KP + (kt + 1) * 128]
                nc.tensor.matmul(
                    pl[:, local:local + 512],
                    lhsT, qall[:, hq + qh:hq + qh + 512],
                    start=True, stop=True)
            nc.scalar.activation(
                ring[:, start:start + size], pl[:, 0:size], Exp, scale=SCALE)

        def emit_av_kt(h, ring, kt, po):
            first, last = kt == 0, kt == kt_tiles - 1
            rhs = vall[:, kt * VW + h * 129: kt * VW + (h + 1) * 129]
            for qi in range(NQT):
                off = po_off(qi)
                # start=True clears the has_written bits of the WHOLE
                # bank, so only the first matmul touching each bank may
                # carry it; the other regions' first writes rely on
                # their (now cleared) bits selecting overwrite mode.
                nc.tensor.matmul(
                    po[:, off:off + 129],
                    ring[:, kt * 1024 + qi * 128: kt * 1024 + (qi + 1) * 128],
                    rhs, start=first and qi % 3 == 0, stop=last,
                    skip_group_check=True)

        def emit_epilogue(h, po):
            # Split per PSUM bank so banks release early: copy to fp16
            # SBUF, reciprocal of the den columns, per-q-tile scalar mul.
            oal = sb_o.tile([128, 1536], F16, tag="oal", name=f"oal_{h}")
            rst = sb_o.tile([128, 9], F32, tag="rst", name=f"rst_{h}")
            osb = sb_o.tile([128, S], F16, tag="osb", name=f"osb_{h}")
            for g in range(3):
                cnt = 3 if g < 3 - 1 else NQT - 6
                base = g * 512
                nc.vector.tensor_copy(
                    oal[:, base:base + cnt * 129], po[:, base:base + cnt * 129])
                nc.vector.reciprocal(
                    rst[:, g * 3:g * 3 + cnt],
                    oal[:, base + 128:base + cnt * 129:129])
                for j in range(cnt):
                    qi = g * 3 + j
                    nc.vector.tensor_scalar_mul(
                        osb[:, qi * 128:(qi + 1) * 128],
                        oal[:, base + j * 129:base + j * 129 + 128],
                        rst[:, qi:qi + 1])
            nc.gpsimd.dma_start(out_t.ap()[h], osb[:])

        rings, pos = {}, {}
        for h in range(H):
            rings[h] = sb_w.tile([128, TOT], F16, tag="ring", name=f"ring_{h}")
            if h >= 1:
                pos[h - 1] = ps_o.tile([128, 1536], F32, tag="po",
                                       name=f"po_{h - 1}")
            for gi in range(len(groups)):
                emit_qk_group(h, gi, rings[h])
                if h >= 1:
                    for kt in kt_sets[gi]:
                        emit_av_kt(h - 1, rings[h - 1], kt, pos[h - 1])
            if h >= 1:
                emit_epilogue(h - 1, pos.pop(h - 1))
                rings.pop(h - 1)
        # Last head: its AV matmuls chase the exps straight down the queue.
        pos[H - 1] = ps_o.tile([128, 1536], F32, tag="po", name=f"po_{H - 1}")
        for kt in range(kt_tiles):
            emit_av_kt(H - 1, rings[H - 1], kt, pos[H - 1])
        emit_epilogue(H - 1, pos.pop(H - 1))

    nc.compile()
    return nc


def kernel(memory, query, seq_mask, b):
    memory = np.ascontiguousarray(memory, dtype=np.float32)
    query = np.ascontiguousarray(query, dtype=np.float32)
    seq_mask = np.asarray(seq_mask)
    assert memory.shape == (B, S, 2 * D) and query.shape == (B, S, D)

    counts = [int(np.count_nonzero(seq_mask[i])) for i in range(B)]
    kp = max(max(counts), 1)
    kp = ((kp + 127) // 128) * 128
    kt_tiles = kp // 128

    key = (kt_tiles, tuple(sorted(OPTS.items())))
    if key not in _NC_CACHE:
        _NC_CACHE[key] = _build(kt_tiles, OPTS)
    nc = _NC_CACHE[key]

    q_t = np.ascontiguousarray(query.transpose(0, 2, 1)).astype(np.float16)
    in_maps = []
    for i in range(B):
        idx = np.flatnonzero(seq_mask[i])
        nb = len(idx)
        ktb = np.zeros((D, kp), dtype=np.float16)
        vab = np.zeros((kp, H, 129), dtype=np.float16)
        if nb:
            ktb[:, :nb] = memory[i, idx, :D].T
            vab[:nb, :, :128] = memory[i, idx, D:].reshape(nb, H, DH)
            vab[:nb, :, 128] = 1.0
        in_maps.append(
            {"q_t": q_t[i], "k_t": ktb, "v_a": vab.reshape(kp, H * 129)})

    res = run_bass_kernel_spmd(nc, in_maps, list(range(B)))
    out = np.empty((B, S, D), dtype=np.float32)
    for i in range(B):
        o = res.results[i]["out_t"].astype(np.float32)   # [H, 128, S]
        # [h, p, (qi d)] -> [qi, p, h, d] -> [S, D]
        out[i] = o.reshape(H, 128, NQT, DH).transpose(2, 1, 0, 3).reshape(S, D)
        if counts[i] == 0:
            # all keys masked: reference softmax degenerates to uniform
            out[i] = memory[i, :, D:].mean(axis=0)[None, :]
    return out


# revision 13
# speedup vs baseline: 1.2960x; 1.2960x over previous
"""Multi-head attention (B=8, H=8, S=1024, d=128) on 8 TRN2 NeuronCores.

Strategy
--------
- Data-parallel over batch: core i handles batch i (8 cores, B=8).
- Host-side prep (layout only): per batch, compact keys/values to the
  seq_mask-selected rows (zero-padded to a multiple of 128 -> kt_tiles
  k-tiles), pre-transpose Q and compacted K so the contraction dim (d)
  lands on SBUF partitions, and cast matmul operands to fp16. V is
  augmented per head with a 129th "indicator" column (1 for real keys,
  0 for padding) so the softmax denominator falls out of the AV matmul.
- Device math per head h:
    logitsT[k, q] = K_h^T.T @ Q_h^T          (PE, M=128 k-tiles, N=512)
    W^T[k, q]     = exp(logitsT * d^-0.5)    (ACT, PSUM -> SBUF fp16,
                                              batched in alternating
                                              1536/1024-col groups to
                                              amortize ACTIVATE overhead)
    out[q, 129]   = sum_kt W^T[kt,qtile].T @ [V_h[kt] | ind[kt]]
                                             (PE, M=128 q-tiles, N=129,
                                              PSUM accumulation over kt;
                                              col 128 = denominator)
    osb[q, d]     = out[:, :128] * recip(out[:, 128])  (DVE)
  The learned scalar bias b cancels in softmax (shift invariance) and
  the -1e30 masking is equivalent to dropping masked keys, which the
  compaction does exactly.
- Software pipelining: AV+epilogue of head h-1 are emitted after QK of
  head h so the scalar engine (the bottleneck at ~42us) never waits at
  head boundaries. A short burst of dummy matmuls at kernel start warms
  the PE HAM clock gate while the first input DMAs are in flight.
- Output per head is DMA'd as a contiguous [128, 1024] fp16 block
  ([q-within-tile, (q-tile, d)]); the host reassembles [S, D] and
  handles the degenerate all-masked batch (uniform average).
"""
from contextlib import ExitStack

import numpy as np

import concourse.bacc as bacc
import concourse.mybir as mybir
import concourse.tile as tile
from concourse.bass_utils import run_bass_kernel_spmd

F32 = mybir.dt.float32
F16 = mybir.dt.float16
Exp = mybir.ActivationFunctionType.Exp
Copy = mybir.ActivationFunctionType.Copy

B, S, D, H = 8, 1024, 1024, 8
DH = D // H              # 128, head dim = one partition tile
SCALE = float(DH) ** -0.5
NQT = S // 128           # 8 q-tiles per head

_NC_CACHE: dict[tuple, object] = {}

# build options (overridable for profiling experiments)
OPTS: dict = {}


def _exp_groups(total):
    """Alternating 1536/1024-col exp groups covering `total` columns."""
    groups, pos, a = [], 0, True
    while pos < total:
        size = min(1536 if a else 1024, total - pos)
        groups.append((a, pos, size))
        pos += size
        a = not a
    return groups


def _build(kt_tiles: int, opts: dict | None = None):
    """Build + compile the per-core kernel for `kt_tiles` 128-wide key tiles."""
    opts = opts or {}
    w_bufs = opts.get("w_bufs", 2)
    o_bufs = opts.get("o_bufs", 2)
    n_warm = opts.get("n_warm", 5)
    KP = kt_tiles * 128
    VW = H * 129             # per-k-row width of augmented V
    TOT = kt_tiles * 1024    # logits columns per head
    groups = _exp_groups(TOT)
    nc = bacc.Bacc("TRN2", target_bir_lowering=False, debug=False)

    q_t = nc.dram_tensor("q_t", [D, S], F16, kind="ExternalInput")
    k_t = nc.dram_tensor("k_t", [D, KP], F16, kind="ExternalInput")
    v_a = nc.dram_tensor("v_a", [KP, VW], F16, kind="ExternalInput")
    out_t = nc.dram_tensor("out_t", [H, 128, S], F16, kind="ExternalOutput")

    # ps_out layout: 3 bank-aligned groups of q-tiles (3+3+2), each q-tile
    # owning 129 columns (128 dims + denominator). Offsets within the
    # [128, 1536] tile; a 129-wide matmul output may not cross a PSUM bank.
    def po_off(qi):
        g, j = divmod(qi, 3)
        return g * 512 + j * 129

    with tile.TileContext(nc) as tc, ExitStack() as ctx:
        sb_k = ctx.enter_context(tc.tile_pool(name="sb_k", bufs=1))
        sb_q = ctx.enter_context(tc.tile_pool(name="sb_q", bufs=1))
        sb_v = ctx.enter_context(tc.tile_pool(name="sb_v", bufs=1))
        sb_wm = ctx.enter_context(tc.tile_pool(name="sb_wm", bufs=1))
        sb_w = ctx.enter_context(tc.tile_pool(name="sb_w", bufs=w_bufs))
        sb_o = ctx.enter_context(tc.tile_pool(name="sb_o", bufs=o_bufs))
        ps_a = ctx.enter_context(tc.tile_pool(name="ps_a", bufs=1, space="PSUM"))
        ps_b = ctx.enter_context(tc.tile_pool(name="ps_b", bufs=1, space="PSUM"))
        ps_o = ctx.enter_context(tc.tile_pool(name="ps_o", bufs=1, space="PSUM"))

        kall = sb_k.tile([128, H * KP], F16)
        qall = sb_q.tile([128, H * S], F16)
        vall = sb_v.tile([128, kt_tiles * VW], F16)

        # --- PE warmup: dense dummy matmuls while the first DMAs fly, so
        # the HAM clock gate reaches 8/8 before real work arrives.
        if n_warm:
            wl = sb_wm.tile([128, 128], F16)
            wr = sb_wm.tile([128, 512], F16)
            nc.gpsimd.memset(wl[:], 0.0)
            nc.gpsimd.memset(wr[:], 0.0)
            warm_po = ps_o.tile([128, 1536], F32, tag="po", name="po_warm")
            for _ in range(n_warm):
                nc.tensor.matmul(warm_po[:, 0:512], wl[:], wr[:],
                                 start=True, stop=True, skip_group_check=True)

        # --- Input DMAs, split per head / per k-tile; the first few are
        # issued from different engines so their transfers overlap.
        def dma_k(h, eng):
            eng.dma_start(
                kall[:, h * KP:(h + 1) * KP], k_t.ap()[h * DH:(h + 1) * DH, :])

        def dma_q(h, eng):
            eng.dma_start(
                qall[:, h * S:(h + 1) * S], q_t.ap()[h * DH:(h + 1) * DH, :])

        def dma_v(kt, eng):
            # straight 2D copy, 2064B rows
            eng.dma_start(
                vall[:, kt * VW:(kt + 1) * VW],
                v_a.ap()[kt * 128:(kt + 1) * 128, :])

        # DMA issue engines: sync/gpsimd/scalar can initiate DMAs. Head 0's
        # first exp group needs k-tiles 0+1 and all of q0, so those three
        # go out in parallel on different engines.
        dma_q(0, nc.sync)
        dma_k(0, nc.gpsimd)
        if kt_tiles > 1:
            dma_k(1, nc.scalar)
        dma_v(0, nc.scalar)
        if kt_tiles > 1:
            dma_v(1, nc.scalar)
        if kt_tiles > 2:
            dma_k(2, nc.gpsimd)
        for kt in range(2, kt_tiles):
            dma_v(kt, nc.gpsimd)
        dma_q(1, nc.sync); dma_q(2, nc.sync)
        for h in range(3, H):
            dma_k(h, nc.gpsimd); dma_q(h, nc.sync)

        def emit_qk_group(h, gi, ring):
            a, start, size = groups[gi]
            hq = h * S
            pool = ps_a if a else ps_b
            cap = 1536 if a else 1024
            pl = pool.tile([128, cap], F32, tag="pl" + ("A" if a else "B"),
                           name=f"pl_{h}_{start}")
            for local in range(0, size, 512):
                gcol = start + local
                kt, qh = divmod(gcol, 1024)
                lhsT = kall[:, h * KP + kt * 128: h * KP + (kt + 1) * 128]
                nc.tensor.matmul(
                    pl[:, local:local + 512],
                    lhsT, qall[:, hq + qh:hq + qh + 512],
                    start=True, stop=True)
            nc.scalar.activation(
                ring[:, start:start + size], pl[:, 0:size], Exp, scale=SCALE)

        def emit_av_kt(h, ring, kt, po):
            first, last = kt == 0, kt == kt_tiles - 1
            rhs = vall[:, kt * VW + h * 129: kt * VW + (h + 1) * 129]
            for qi in range(NQT):
                off = po_off(qi)
                # start=True clears the has_written bits of the WHOLE
                # bank, so only the first matmul touching each bank may
                # carry it; the other regions' first writes rely on
                # their (now cleared) bits selecting overwrite mode.
                nc.tensor.matmul(
                    po[:, off:off + 129],
                    ring[:, kt * 1024 + qi * 128: kt * 1024 + (qi + 1) * 128],
                    rhs, start=first and qi % 3 == 0, stop=last,
                    skip_group_check=True)

        def emit_epilogue(h, po):
            # Split per PSUM bank so banks release early: copy to fp16
            # SBUF, reciprocal of the den columns, per-q-tile scalar mul.
            oal = sb_o.tile([128, 1536], F16, tag="oal", name=f"oal_{h}")
            rst = sb_o.tile([128, 9], F32, tag="rst", name=f"rst_{h}")
            osb = sb_o.tile([128, S], F16, tag="osb", name=f"osb_{h}")
            # copies first: they are what releases the po PSUM banks
            for g in range(3):
                cnt = 3 if g < 3 - 1 else NQT - 6
                base = g * 512
                nc.vector.tensor_copy(
                    oal[:, base:base + cnt * 129], po[:, base:base + cnt * 129])
            for g in range(3):
                cnt = 3 if g < 3 - 1 else NQT - 6
                base = g * 512
                nc.vector.reciprocal(
                    rst[:, g * 3:g * 3 + cnt],
                    oal[:, base + 128:base + cnt * 129:129])
            for qi in range(NQT):
                g, j = divmod(qi, 3)
                base = g * 512
                nc.vector.tensor_scalar_mul(
                    osb[:, qi * 128:(qi + 1) * 128],
                    oal[:, base + j * 129:base + j * 129 + 128],
                    rst[:, qi:qi + 1])
            nc.gpsimd.dma_start(out_t.ap()[h], osb[:])

        # Boundary-level software pipeline. Per head h the PE queue gets:
        #   QK(h, g0) | AV(h-1, kt 0..last-1) | QK(h, g1) | AV(h-1, last kt)
        #   | QK(h, g2..) | epilogue(h-1)
        # so exp(h, g0) starts the moment exp(h-1, last) finishes, and the
        # previous head's AV (whose exps are long done) fills PE idle time
        # without head-of-line blocking the QK stream.
        rings, pos = {}, {}
        for h in range(H):
            rings[h] = sb_w.tile([128, TOT], F16, tag="ring", name=f"ring_{h}")
            if h >= 1:
                pos[h - 1] = ps_o.tile([128, 1536], F32, tag="po",
                                       name=f"po_{h - 1}")
            for gi in range(len(groups)):
                emit_qk_group(h, gi, rings[h])
                if h >= 1 and gi == 0:
                    for kt in range(kt_tiles - 1):
                        emit_av_kt(h - 1, rings[h - 1], kt, pos[h - 1])
                if h >= 1 and gi == 1:
                    emit_av_kt(h - 1, rings[h - 1], kt_tiles - 1, pos[h - 1])
            if h >= 1:
                emit_epilogue(h - 1, pos.pop(h - 1))
                rings.pop(h - 1)
        # Last head: its AV matmuls chase the exps straight down the queue.
        pos[H - 1] = ps_o.tile([128, 1536], F32, tag="po", name=f"po_{H - 1}")
        for kt in range(kt_tiles):
            emit_av_kt(H - 1, rings[H - 1], kt, pos[H - 1])
        emit_epilogue(H - 1, pos.pop(H - 1))

    nc.compile()
    return nc


def kernel(memory, query, seq_mask, b):
    memory = np.ascontiguousarray(memory, dtype=np.float32)
    query = np.ascontiguousarray(query, dtype=np.float32)
    seq_mask = np.asarray(seq_mask)
    assert memory.shape == (B, S, 2 * D) and query.shape == (B, S, D)

    counts = [int(np.count_nonzero(seq_mask[i])) for i in range(B)]
    kp = max(max(counts), 1)
    kp = ((kp + 127) // 128) * 128
    kt_tiles = kp // 128

    key = (kt_tiles, tuple(sorted(OPTS.items())))
    if key not in _NC_CACHE:
        _NC_CACHE[key] = _build(kt_tiles, OPTS)
    nc = _NC_CACHE[key]

    q_t = np.ascontiguousarray(query.transpose(0, 2, 1)).astype(np.float16)
    in_maps = []
    for i in range(B):
        idx = np.flatnonzero(seq_mask[i])
        nb = len(idx)
        ktb = np.zeros((D, kp), dtype=np.float16)
        vab = np.zeros((kp, H, 129), dtype=np.float16)
        if nb:
            ktb[:, :nb] = memory[i, idx, :D].T
            vab[:nb, :, :128] = memory[i, idx, D:].reshape(nb, H, DH)
            vab[:nb, :, 128] = 1.0
        in_maps.append(
            {"q_t": q_t[i], "k_t": ktb, "v_a": vab.reshape(kp, H * 129)})

    res = run_bass_kernel_spmd(nc, in_maps, list(range(B)))
    out = np.empty((B, S, D), dtype=np.float32)
    for i in range(B):
        o = res.results[i]["out_t"].astype(np.float32)   # [H, 128, S]
        # [h, p, (qi d)] -> [qi, p, h, d] -> [S, D]
        out[i] = o.reshape(H, 128, NQT, DH).transpose(2, 1, 0, 3).reshape(S, D)
        if counts[i] == 0:
            # all keys masked: reference softmax degenerates to uniform
            out[i] = memory[i, :, D:].mean(axis=0)[None, :]
    return out
